# revision 1
# baseline (speedup 1.0000x reference)
"""CTDGConv Bass kernel for 8 TRN2 NeuronCores.

Strategy (self-contained; shapes hardcoded for N=50000, E=800000):
- Host: permute nodes to balance dst-degree over 128-node blocks; shard edges
  across 8 cores by dst-node-range; sort/pad edges into (block, tile) slots.
- Device (SPMD, one program): precompute node tables Tm=[x@Wm1+bm | x@Wm1Wa1+..]
  -> per-edge-tile: stream edge features, gather Tm[src] (indirect DMA),
  msgs = epart + Tm_g; e1 = exp(l1-part); scatter via one-hot matmul into
  per-(dst-block, head) PSUM accumulators; s1 = per-node sum of e1.
  AllReduce 4-number softmax denominator; apply node-level q/(4D) scaling;
  final MLP (exact gelu) + LayerNorm on own nodes.
"""
import heapq
import numpy as np

N, E = 50000, 800000
NODE_DIM, EDGE_DIM, TIME_DIM, OUT_DIM, N_HEADS = 128, 64, 16, 128, 4
NC_CORES = 8
NPC = N // NC_CORES          # 6250
PB = 128
NBLK = (NPC + PB - 1) // PB  # 49
LAST_BLK = NPC - (NBLK - 1) * PB  # 106
NPAD = NC_CORES * NBLK * PB  # 50176
ECDIM = EDGE_DIM + TIME_DIM  # 80


# ---------------------------------------------------------------- host prep
def _host_prep(x, edge_index, edge_feat, time_deltas, gammas, Wm, bm, Wa, ba):
    src = np.asarray(edge_index[0]).astype(np.int64)
    dst = np.asarray(edge_index[1]).astype(np.int64)
    deg = np.bincount(dst, minlength=N)

    n_bins = NC_CORES * NBLK
    cap = np.full(n_bins, PB, np.int64)
    cap[NBLK - 1::NBLK] = LAST_BLK
    order = np.argsort(-deg, kind="stable")
    heap = [(0, b) for b in range(n_bins)]
    heapq.heapify(heap)
    bin_of_node = np.empty(N, np.int64)
    bin_fill = np.zeros(n_bins, np.int64)
    for n in order:
        while True:
            cnt, b = heapq.heappop(heap)
            if bin_fill[b] < cap[b]:
                break
        bin_of_node[n] = b
        bin_fill[b] += 1
        if bin_fill[b] < cap[b]:
            heapq.heappush(heap, (cnt + deg[n], b))
    bin_base = (np.arange(n_bins) // NBLK) * NPC + (np.arange(n_bins) % NBLK) * PB
    pos_of = np.empty(N, np.int64)
    fill2 = np.zeros(n_bins, np.int64)
    for n in np.argsort(bin_of_node, kind="stable"):
        b = bin_of_node[n]
        pos_of[n] = bin_base[b] + fill2[b]
        fill2[b] += 1
    perm_to_orig = np.empty(N, np.int64)
    perm_to_orig[pos_of] = np.arange(N)

    srcp = pos_of[src]
    dstp = pos_of[dst]
    core_of_e = dstp // NPC
    blk_of_e = (dstp % NPC) // PB
    dst_rel = (dstp % NPC) - blk_of_e * PB
    bin_of_e = core_of_e * NBLK + blk_of_e

    cnt = np.bincount(bin_of_e, minlength=n_bins)
    T_blk = int(np.ceil(cnt.max() / 128))
    T_blk += T_blk % 2  # even, for possible grouping
    EPT = T_blk * 128
    E_cap = NBLK * EPT

    order_e = np.argsort(bin_of_e, kind="stable")
    starts = np.zeros(n_bins + 1, np.int64)
    np.cumsum(cnt, out=starts[1:])
    e_sorted = order_e
    b_sorted = bin_of_e[e_sorted]
    idx_within = np.arange(E) - starts[b_sorted]
    k_sorted = b_sorted // NBLK
    blk_sorted = b_sorted % NBLK
    pos_in_core = blk_sorted * EPT + idx_within

    srcp_pad = np.zeros((NC_CORES, E_cap), np.int32)
    dst_rel_pad = np.zeros((NC_CORES, E_cap), np.float32)
    mask_pad = np.zeros((NC_CORES, E_cap), np.float32)
    ef_pad = np.zeros((NC_CORES, E_cap, EDGE_DIM), np.float32)
    dt_pad = np.zeros((NC_CORES, E_cap), np.float32)
    srcp_pad[k_sorted, pos_in_core] = srcp[e_sorted].astype(np.int32)
    dst_rel_pad[k_sorted, pos_in_core] = dst_rel[e_sorted].astype(np.float32)
    mask_pad[k_sorted, pos_in_core] = 1.0
    ef_pad[k_sorted, pos_in_core] = np.asarray(edge_feat, np.float32)[e_sorted]
    dt_pad[k_sorted, pos_in_core] = np.asarray(time_deltas, np.float32)[e_sorted]

    Wm1 = np.asarray(Wm[:NODE_DIM], np.float32)
    Wm2 = np.asarray(Wm[NODE_DIM:], np.float32)
    Wa1 = np.asarray(Wa[:OUT_DIM], np.float32)
    Wa2 = np.asarray(Wa[OUT_DIM:], np.float32)
    Wv1 = Wm1 @ Wa1
    Wv1e = Wm2 @ Wa1
    bv1a = np.asarray(bm, np.float32) @ Wa1 + np.asarray(ba, np.float32)
    neg_spg = -np.log1p(np.exp(np.asarray(gammas, np.float32)))
    x_perm = np.asarray(x, np.float32)[perm_to_orig]

    return dict(pos_of=pos_of, T_blk=T_blk, EPT=EPT, E_cap=E_cap,
                srcp_pad=srcp_pad, dst_rel_pad=dst_rel_pad, mask_pad=mask_pad,
                ef_pad=ef_pad, dt_pad=dt_pad, Wm1=Wm1, Wm2=Wm2, Wa2=Wa2,
                Wv1=Wv1, Wv1e=Wv1e, bv1a=bv1a, neg_spg=neg_spg, x_perm=x_perm)


# ---------------------------------------------------------------- program
_PROGRAM_CACHE = {}


def _build_program(T_blk, timing_reps=1):
    from concourse import bass, mybir, bacc
    import concourse.tile as tile
    from concourse.masks import make_identity

    f32 = mybir.dt.float32
    bf16 = mybir.dt.bfloat16
    i32 = mybir.dt.int32
    AF = mybir.ActivationFunctionType
    OP = mybir.AluOpType

    EPT = T_blk * 128
    E_cap = NBLK * EPT

    nc = bacc.Bacc("TRN2", target_bir_lowering=False, debug=False,
                   num_devices=NC_CORES)
    # inputs
    xT = nc.dram_tensor("xT", [128, NPAD], f32, kind="ExternalInput")
    xoT = nc.dram_tensor("xoT", [128, NBLK * PB], f32, kind="ExternalInput")
    ef = nc.dram_tensor("ef", [EDGE_DIM, E_cap], bf16, kind="ExternalInput")
    dtv = nc.dram_tensor("dtv", [1, E_cap], f32, kind="ExternalInput")
    G_EF = max(g for g in (8, 6, 4, 3, 2, 1) if T_blk % g == 0)
    n_groups = NBLK * (T_blk // G_EF)
    meta = nc.dram_tensor("meta", [n_groups * 128, 4 * G_EF], i32,
                          kind="ExternalInput")
    Wm1 = nc.dram_tensor("Wm1", [128, 128], f32, kind="ExternalInput")
    Wv1 = nc.dram_tensor("Wv1", [128, 4], f32, kind="ExternalInput")
    Wa2 = nc.dram_tensor("Wa2", [128, 4], f32, kind="ExternalInput")
    Wm2b = nc.dram_tensor("Wm2b", [ECDIM, 128], bf16, kind="ExternalInput")
    Wv1eb = nc.dram_tensor("Wv1eb", [ECDIM, 4], bf16, kind="ExternalInput")
    Wo1 = nc.dram_tensor("Wo1", [128, 128], f32, kind="ExternalInput")
    Wo2 = nc.dram_tensor("Wo2", [128, 128], f32, kind="ExternalInput")
    bmv = nc.dram_tensor("bmv", [128, 1], f32, kind="ExternalInput")
    bv1a = nc.dram_tensor("bv1a", [4, 1], f32, kind="ExternalInput")
    bov = nc.dram_tensor("bov", [128, 1], f32, kind="ExternalInput")
    lng = nc.dram_tensor("lng", [128, 1], f32, kind="ExternalInput")
    lnb = nc.dram_tensor("lnb", [128, 1], f32, kind="ExternalInput")
    nspg = nc.dram_tensor("nspg", [TIME_DIM, 1], f32, kind="ExternalInput")
    out_d = nc.dram_tensor("out", [NPC, 128], f32, kind="ExternalOutput")
    # internal DRAM
    Tm = nc.dram_tensor("Tm", [NPAD, 132], f32)
    Qd = nc.dram_tensor("Qd", [NBLK, PB * N_HEADS], f32)

    with tile.TileContext(nc) as tc:
        with tc.tile_pool(name="const", bufs=1) as cpool, \
             tc.tile_pool(name="pers", bufs=1) as ppool, \
             tc.tile_pool(name="work", bufs=3) as pool, \
             tc.tile_pool(name="psum", bufs=2, space="PSUM") as psum, \
             tc.tile_pool(name="dram", bufs=1, space="DRAM") as dram:

            # ---------------- constants
            ident = cpool.tile([128, 128], f32)
            make_identity(nc, ident[:])
            iota_i32 = cpool.tile([128, 512], i32)
            nc.gpsimd.iota(iota_i32[:, :], pattern=[[1, 128], [0, 4]], base=0,
                           channel_multiplier=0)
            iota_j4 = cpool.tile([128, 512], bf16)
            nc.vector.tensor_copy(out=iota_j4[:], in_=iota_i32[:])
            iota_j_i32 = cpool.tile([128, 128], i32)
            nc.gpsimd.iota(iota_j_i32[:, :], pattern=[[1, 128]], base=0,
                           channel_multiplier=0)
            iota_j = cpool.tile([128, 128], bf16)
            nc.vector.tensor_copy(out=iota_j[:], in_=iota_j_i32[:])
            ones_col = cpool.tile([128, 1], f32)
            nc.vector.memset(ones_col[:], 1.0)
            eps_col = cpool.tile([128, 1], f32)
            nc.vector.memset(eps_col[:], 1e-5)

            w_Wm1 = cpool.tile([128, 128], f32)
            nc.sync.dma_start(out=w_Wm1[:], in_=Wm1[:, :])
            w_Wv1 = cpool.tile([128, 4], f32)
            nc.sync.dma_start(out=w_Wv1[:], in_=Wv1[:, :])
            w_Wa2 = cpool.tile([128, 4], f32)
            nc.sync.dma_start(out=w_Wa2[:], in_=Wa2[:, :])
            w_Wm2b = cpool.tile([ECDIM, 128], bf16)
            nc.sync.dma_start(out=w_Wm2b[:], in_=Wm2b[:, :])
            w_Wv1eb = cpool.tile([ECDIM, 4], bf16)
            nc.sync.dma_start(out=w_Wv1eb[:], in_=Wv1eb[:, :])
            w_Wo1 = cpool.tile([128, 128], f32)
            nc.sync.dma_start(out=w_Wo1[:], in_=Wo1[:, :])
            w_Wo2 = cpool.tile([128, 128], f32)
            nc.sync.dma_start(out=w_Wo2[:], in_=Wo2[:, :])
            c_nspg = cpool.tile([TIME_DIM, 1], f32)
            nc.sync.dma_start(out=c_nspg[:], in_=nspg[:, :])

            # row-replicated constants via PE transpose of broadcast columns
            def rep_row(name, dram_col, width):
                col = cpool.tile([width, 1], f32, tag=f"rc_{name}")
                nc.sync.dma_start(out=col[:], in_=dram_col[:, :])
                ps = psum.tile([128, width], f32, tag="blk")
                nc.tensor.transpose(out=ps[:], in_=col[:].to_broadcast([width, 128]),
                                    identity=ident[0:width, 0:width])
                rep = cpool.tile([128, width], f32, tag=f"rep_{name}")
                nc.vector.tensor_copy(out=rep[:], in_=ps[:])
                return rep

            bm_rep = rep_row("bm", bmv, 128)
            bv1a_rep = rep_row("bv1a", bv1a, 4)
            bo_rep = rep_row("bo", bov, 128)
            lng_rep = rep_row("lng", lng, 128)
            lnb_rep = rep_row("lnb", lnb, 128)

            # ---------------- persistent
            agg1_all = ppool.tile([128, NBLK * PB * N_HEADS], bf16)
            s1_all = ppool.tile([128, NBLK * N_HEADS], f32)
            q_all = ppool.tile([128, NBLK * N_HEADS], f32)
            d_acc = ppool.tile([N_HEADS, 1], f32)
            dinv_rep = ppool.tile([128, N_HEADS], f32)

            def phase_P():
                for c in range(0, NPAD, 128):
                    xt = pool.tile([128, 128], f32, tag="xt")
                    nc.sync.dma_start(out=xt[:], in_=xT[:, c:c + 128])
                    ps = psum.tile([128, 132], f32, tag="ml")
                    nc.tensor.matmul(out=ps[:, 0:128], lhsT=xt[:], rhs=w_Wm1[:],
                                     start=True, stop=True)
                    nc.tensor.matmul(out=ps[:, 128:132], lhsT=xt[:], rhs=w_Wv1[:],
                                     start=True, stop=True)
                    tm_sb = pool.tile([128, 132], f32, tag="tm")
                    nc.vector.tensor_tensor(out=tm_sb[:, 0:128], in0=ps[:, 0:128],
                                            in1=bm_rep[:], op=OP.add)
                    nc.vector.tensor_tensor(out=tm_sb[:, 128:132], in0=ps[:, 128:132],
                                            in1=bv1a_rep[:], op=OP.add)
                    nc.scalar.dma_start(out=Tm[c:c + 128, :], in_=tm_sb[:])

            def phase_edges():
                nc.vector.memset(d_acc[:], 0.0)
                for b in range(NBLK):
                    ps_blk = psum.tile([128, 512], f32, tag="blk")
                    ps_s1 = psum.tile([128, 4], f32, tag="s1")
                    for tg in range(T_blk // G_EF):
                        GW = G_EF * 128
                        e0g = b * EPT + tg * GW
                        gidx = b * (T_blk // G_EF) + tg
                        efte_g = pool.tile([ECDIM, GW], bf16, tag="efte")
                        nc.scalar.dma_start(out=efte_g[0:EDGE_DIM, :],
                                            in_=ef[:, e0g:e0g + GW])
                        dt_g = pool.tile([TIME_DIM, GW], f32, tag="dt")
                        nc.scalar.dma_start(
                            out=dt_g[:],
                            in_=dtv[0:1, e0g:e0g + GW].to_broadcast([TIME_DIM, GW]))
                        nc.scalar.activation(out=efte_g[EDGE_DIM:ECDIM, :],
                                             in_=dt_g[:], func=AF.Exp,
                                             scale=c_nspg[:, 0:1])
                        meta_t = pool.tile([128, 4 * G_EF], i32, tag="meta")
                        nc.sync.dma_start(out=meta_t[:],
                                          in_=meta[gidx * 128:(gidx + 1) * 128, :])
                        for c in range(G_EF):
                            t = tg * G_EF + c
                            efte = efte_g[:, c * 128:(c + 1) * 128]
                            srct = meta_t[:, 4 * c:4 * c + 1]
                            dstt = meta_t[:, 4 * c + 1:4 * c + 2].bitcast(f32)
                            maskt = meta_t[:, 4 * c + 2:4 * c + 3].bitcast(f32)
                            g = pool.tile([128, 132], f32, tag="g")
                            nc.gpsimd.indirect_dma_start(
                                out=g[:], out_offset=None, in_=Tm[:],
                                in_offset=bass.IndirectOffsetOnAxis(ap=srct, axis=0))
                            ps_ml = psum.tile([128, 132], f32, tag="ml")
                            nc.tensor.matmul(out=ps_ml[:, 0:128], lhsT=efte,
                                             rhs=w_Wm2b[:], start=True, stop=True)
                            nc.tensor.matmul(out=ps_ml[:, 128:132], lhsT=efte,
                                             rhs=w_Wv1eb[:], start=True, stop=True)
                            msgs = pool.tile([128, 128], bf16, tag="msgs")
                            nc.vector.tensor_tensor(out=msgs[:], in0=ps_ml[:, 0:128],
                                                    in1=g[:, 0:128], op=OP.add)
                            l1t = pool.tile([128, 4], f32, tag="l1t")
                            nc.vector.tensor_tensor(out=l1t[:], in0=ps_ml[:, 128:132],
                                                    in1=g[:, 128:132], op=OP.add)
                            e1 = pool.tile([128, 4], f32, tag="e1")
                            nc.scalar.activation(out=e1[:], in_=l1t[:], func=AF.Exp)
                            e1m = pool.tile([128, 4], bf16, tag="e1m")
                            nc.vector.tensor_scalar(out=e1m[:], in0=e1[:],
                                                    scalar1=maskt, scalar2=None,
                                                    op0=OP.mult)
                            s_pre = pool.tile([128, 512], bf16, tag="s_pre")
                            nc.vector.tensor_scalar(out=s_pre[:], in0=iota_j4[:],
                                                    scalar1=dstt, scalar2=None,
                                                    op0=OP.is_equal)
                            s_small = pool.tile([128, 128], bf16, tag="s_small")
                            nc.vector.tensor_scalar(out=s_small[:], in0=iota_j[:],
                                                    scalar1=dstt, scalar2=None,
                                                    op0=OP.is_equal)
                            s_ext = pool.tile([128, 512], bf16, tag="s_ext")
                            nc.vector.tensor_tensor(
                                out=s_ext[:], in0=s_pre[:],
                                in1=e1m[:, None, :].to_broadcast([128, 128, 4]),
                                op=OP.mult)
                            nc.tensor.matmul(out=ps_blk[:], lhsT=msgs[:], rhs=s_ext[:],
                                             start=(t == 0), stop=(t == T_blk - 1),
                                             skip_group_check=True)
                            nc.tensor.matmul(out=ps_s1[:], lhsT=s_small[:], rhs=e1m[:],
                                             start=(t == 0), stop=(t == T_blk - 1),
                                             skip_group_check=True)
                    # block epilogue
                    nc.vector.tensor_copy(
                        out=agg1_all[:, b * 512:(b + 1) * 512], in_=ps_blk[:])
                    nc.vector.tensor_copy(
                        out=s1_all[:, b * 4:(b + 1) * 4], in_=ps_s1[:])
                    xo_t = pool.tile([128, 128], f32, tag="xo")
                    nc.sync.dma_start(out=xo_t[:], in_=xoT[:, b * 128:(b + 1) * 128])
                    ps_x = psum.tile([128, 4], f32, tag="epi")
                    nc.tensor.matmul(out=ps_x[:], lhsT=xo_t[:], rhs=w_Wa2[:],
                                     start=True, stop=True, skip_group_check=True)
                    nc.scalar.activation(out=q_all[:, b * 4:(b + 1) * 4], in_=ps_x[:],
                                         func=AF.Exp)
                    qs1 = pool.tile([128, 4], f32, tag="qs1")
                    nc.vector.tensor_tensor(out=qs1[:],
                                            in0=q_all[:, b * 4:(b + 1) * 4],
                                            in1=s1_all[:, b * 4:(b + 1) * 4],
                                            op=OP.mult)
                    ps_d = psum.tile([4, 1], f32, tag="epi")
                    nc.tensor.matmul(out=ps_d[:], lhsT=qs1[:], rhs=ones_col[:],
                                     start=True, stop=True, skip_group_check=True)
                    nc.vector.tensor_tensor(out=d_acc[:], in0=d_acc[:], in1=ps_d[:],
                                            op=OP.add)

            def phase_F():
                for b in range(NBLK):
                    nvalid = PB if b < NBLK - 1 else LAST_BLK
                    q_rep = pool.tile([128, 512], f32, tag="q_rep")
                    nc.sync.dma_start(
                        out=q_rep[:],
                        in_=Qd[b:b + 1, :].to_broadcast([128, PB * N_HEADS]))
                    t1 = pool.tile([128, 128, 4], f32, tag="t1")
                    nc.vector.tensor_tensor(
                        out=t1[:, :, :], in0=agg1_all[:, b * 512:(b + 1) * 512],
                        in1=q_rep[:], op=OP.mult)
                    agg_blk = pool.tile([128, 128], f32, tag="aggb")
                    nc.vector.tensor_reduce(out=agg_blk[:], in_=t1[:, :, :],
                                            axis=mybir.AxisListType.X, op=OP.add)
                    ps_f = psum.tile([128, 128], f32, tag="ml")
                    xo_t = pool.tile([128, 128], f32, tag="xo")
                    nc.sync.dma_start(out=xo_t[:], in_=xoT[:, b * 128:(b + 1) * 128])
                    nc.tensor.matmul(out=ps_f[:], lhsT=agg_blk[:], rhs=w_Wo1[:],
                                     start=True, stop=False, skip_group_check=True)
                    nc.tensor.matmul(out=ps_f[:], lhsT=xo_t[:], rhs=w_Wo2[:],
                                     start=False, stop=True, skip_group_check=True)
                    y = pool.tile([128, 128], f32, tag="y")
                    nc.vector.tensor_tensor(out=y[:], in0=ps_f[:], in1=bo_rep[:],
                                            op=OP.add)
                    ge = pool.tile([128, 128], f32, tag="ge")
                    musum = pool.tile([128, 1], f32, tag="musum")
                    nc.scalar.activation(out=ge[:], in_=y[:], func=AF.Gelu,
                                         accum_out=musum[:])
                    mu = pool.tile([128, 1], f32, tag="mu")
                    nc.scalar.mul(out=mu[:], in_=musum[:], mul=1.0 / 128)
                    z = pool.tile([128, 128], f32, tag="z")
                    nc.vector.tensor_scalar(out=z[:], in0=ge[:], scalar1=mu[:, 0:1],
                                            scalar2=None, op0=OP.subtract)
                    z2 = pool.tile([128, 128], f32, tag="z2")
                    sssum = pool.tile([128, 1], f32, tag="sssum")
                    nc.scalar.activation(out=z2[:], in_=z[:], func=AF.Square,
                                         accum_out=sssum[:])
                    sd = pool.tile([128, 1], f32, tag="sd")
                    nc.scalar.activation(out=sd[:], in_=sssum[:], func=AF.Sqrt,
                                         scale=1.0 / 128, bias=eps_col[:, 0:1])
                    rstd = pool.tile([128, 1], f32, tag="rstd")
                    nc.vector.reciprocal(out=rstd[:], in_=sd[:])
                    o1 = pool.tile([128, 128], f32, tag="o1")
                    nc.vector.tensor_scalar(out=o1[:], in0=z[:], scalar1=rstd[:, 0:1],
                                            scalar2=None, op0=OP.mult)
                    o2 = pool.tile([128, 128], f32, tag="o2")
                    nc.vector.tensor_tensor(out=o2[:], in0=o1[:], in1=lng_rep[:],
                                            op=OP.mult)
                    o3 = pool.tile([128, 128], f32, tag="o3")
                    nc.vector.tensor_tensor(out=o3[:], in0=o2[:], in1=lnb_rep[:],
                                            op=OP.add)
                    nc.scalar.dma_start(out=out_d[b * 128:b * 128 + nvalid, :],
                                        in_=o3[0:nvalid, :])

            # ---------------- main flow
            if timing_reps > 1:
                with tc.For_i(0, timing_reps, 1):
                    phase_P()
                    phase_edges()
            else:
                phase_P()
                phase_edges()

            # AllReduce of D (4 numbers, padded to 8)
            d8 = pool.tile([8, 1], f32, tag="d8")
            nc.vector.memset(d8[:], 0.0)
            nc.vector.tensor_copy(out=d8[0:4, :], in_=d_acc[:])
            din = dram.tile([1, 8], f32)
            dout = dram.tile([1, 8], f32)
            nc.gpsimd.dma_start(out=din[:], in_=d8[:])
            nc.gpsimd.collective_compute(
                "AllReduce", mybir.AluOpType.add,
                replica_groups=[list(range(NC_CORES))],
                ins=[din[:]], outs=[dout[:]])
            dsum = pool.tile([8, 1], f32, tag="dsum")
            nc.gpsimd.dma_start(out=dsum[:], in_=dout[:])
            d4 = pool.tile([4, 1], f32, tag="d4")
            nc.scalar.mul(out=d4[:], in_=dsum[0:4, :], mul=4.0)
            dinv = pool.tile([4, 1], f32, tag="dinv")
            nc.vector.reciprocal(out=dinv[:], in_=d4[:])
            ps_r = psum.tile([128, 4], f32, tag="epi")
            nc.tensor.transpose(out=ps_r[:], in_=dinv[:].to_broadcast([4, 128]),
                                identity=ident[0:4, 0:4])
            nc.vector.tensor_copy(out=dinv_rep[:], in_=ps_r[:])
            # Q blocks -> Qd
            for b in range(NBLK):
                qsb = pool.tile([128, 4], f32, tag="qsb")
                nc.vector.tensor_tensor(out=qsb[:], in0=q_all[:, b * 4:(b + 1) * 4],
                                        in1=dinv_rep[:], op=OP.mult)
                nc.scalar.dma_start(out=Qd[b:b + 1, :], in_=qsb[:])

            if timing_reps > 1:
                with tc.For_i(0, timing_reps, 1):
                    phase_F()
            else:
                phase_F()

    nc.compile()
    return nc


def _get_program(T_blk, timing_reps=1):
    key = (T_blk, timing_reps)
    if key not in _PROGRAM_CACHE:
        _PROGRAM_CACHE[key] = _build_program(T_blk, timing_reps)
    return _PROGRAM_CACHE[key]


# ---------------------------------------------------------------- kernel
def _make_in_maps(prep):
    import ml_dtypes
    bf = ml_dtypes.bfloat16
    E_cap = prep["E_cap"]
    x_perm = prep["x_perm"]
    xT_np = np.zeros((128, NPAD), np.float32)
    xT_np[:, :N] = x_perm.T
    common = {
        "xT": xT_np,
        "Wm1": prep["Wm1"], "Wv1": prep["Wv1"], "Wa2": prep["Wa2"],
        "Wm2b": prep["Wm2"].astype(bf), "Wv1eb": prep["Wv1e"].astype(bf),
        "Wo1": None, "Wo2": None,
        "bmv": None, "bv1a": prep["bv1a"].reshape(4, 1),
        "bov": None, "lng": None, "lnb": None,
        "nspg": prep["neg_spg"].reshape(TIME_DIM, 1),
    }
    in_maps = []
    for k in range(NC_CORES):
        m = dict(common)
        xo = np.zeros((128, NBLK * PB), np.float32)
        xo[:, :NPC] = x_perm[k * NPC:(k + 1) * NPC].T
        m["xoT"] = xo
        m["ef"] = np.ascontiguousarray(prep["ef_pad"][k].T).astype(bf)
        m["dtv"] = prep["dt_pad"][k].reshape(1, E_cap)
        T_blk = prep["T_blk"]
        G_EF = max(gg for gg in (8, 6, 4, 3, 2, 1) if T_blk % gg == 0)
        n_groups = NBLK * (T_blk // G_EF)
        meta = np.zeros((E_cap, 4), np.int32)
        meta[:, 0] = prep["srcp_pad"][k]
        meta[:, 1] = prep["dst_rel_pad"][k].view(np.int32)
        meta[:, 2] = prep["mask_pad"][k].view(np.int32)
        m["meta"] = np.ascontiguousarray(
            meta.reshape(n_groups, G_EF, 128, 4).transpose(0, 2, 1, 3)
        ).reshape(n_groups * 128, 4 * G_EF)
        in_maps.append(m)
    return in_maps


def kernel(x, edge_index, edge_feat, time_deltas, gammas, Wm, bm, Wa, ba,
           Wo, bo, ln_g, ln_b):
    from concourse.bass_utils import run_bass_kernel_spmd

    prep = _host_prep(x, edge_index, edge_feat, time_deltas, gammas, Wm, bm,
                      Wa, ba)
    nc = _get_program(prep["T_blk"])
    in_maps = _make_in_maps(prep)
    fills = {
        "Wo1": np.asarray(Wo, np.float32)[:OUT_DIM],
        "Wo2": np.asarray(Wo, np.float32)[OUT_DIM:],
        "bmv": np.asarray(bm, np.float32).reshape(128, 1),
        "bov": np.asarray(bo, np.float32).reshape(128, 1),
        "lng": np.asarray(ln_g, np.float32).reshape(128, 1),
        "lnb": np.asarray(ln_b, np.float32).reshape(128, 1),
    }
    for m in in_maps:
        m.update(fills)
    res = run_bass_kernel_spmd(nc, in_maps, core_ids=list(range(NC_CORES)))
    out_perm = np.concatenate([res.results[k]["out"] for k in range(NC_CORES)], 0)
    return out_perm[prep["pos_of"]].astype(np.float32)

